# revision 18
# baseline (speedup 1.0000x reference)
"""CombinedMarginLoss (ArcFace m1=1, m2=0.5, m3=0 + interclass filtering) on 8 trn2 cores.

Sharding: batch dim B=1024 split into 8 slabs of 128 rows (one per core), so
every row's target entry is local to the core that owns the row.

The kernel is HBM-bandwidth bound (pure streaming elementwise over
[1024, 100000] f32), so device I/O uses bf16 to halve the traffic:

- Input encode (host, part of sharding): x_bf16 = round(x). The reference
  predicate (x > 0.3 in f32) can flip under bf16 rounding for x in
  (0.29980, 0.3], so those elements are nudged down to T_DEV = 0.298828125
  (the largest bf16 <= 0.3, exactly representable in both bf16 and f32).
  The device compares against T_DEV, which then reproduces the f32
  predicate exactly; the nudge keeps |xb - x| <= 2 ulp (~0.4% rel).
- The ArcFace margin needs the f32 target logit (sqrt(1-t^2) cancels
  catastrophically near t=1 in bf16), so the host passes the 128 gathered
  target values per core as a tiny f32 side input; the device computes the
  margin chain in f32 and returns it as a small f32 output that the host
  scatters into the final f32 result during unshard.

Per-core program (SPMD, same BIR on all 8 cores):
  - elementwise over [128, 100000] bf16: out = (x <= T_DEV) ? 64*x : 0
    (tensor_scalar mask runs in 4x DVE mode, tensor_tensor mult in 2x)
  - margin chain on [128, 1] f32 from the target-value input, stored to a
    [128, 1] f32 output.
"""

import math

import numpy as np
import ml_dtypes

import concourse.bacc as bacc
import concourse.mybir as mybir
import concourse.tile as tile
from concourse.bass_utils import run_bass_kernel_spmd

B, C = 1024, 100000
N_CORES = 8
RB = B // N_CORES  # 128 rows per core == SBUF partition count

S = 64.0
M2 = 0.5
COS_M = math.cos(M2)
SIN_M = math.sin(M2)
THETA = math.cos(math.pi - M2)
SINMM = math.sin(math.pi - M2) * M2

THRESH = np.float32(0.3)  # the reference's f32 predicate constant
BF16 = ml_dtypes.bfloat16
T_DEV = np.float32(0.298828125)  # largest bf16 <= 0.3; bf16- and f32-exact

TF = 5000  # free-dim tile width (10KB/partition per bf16 tile)
LAYOUT = "row"  # "row" or "tile" (tile-major contiguous DRAM blocks)

F32 = mybir.dt.float32
BF = mybir.dt.bfloat16


def make_plan(c, tf, ramp=(1250, 1250, 2500, 5000)):
    """Tile widths: geometric ramp at both ends so the pipeline fills fast
    (small first load -> compute starts early) and drains fast (small last
    store), full-width tiles in the middle."""
    head = list(ramp)
    tail = list(ramp)[::-1]
    mid = c - sum(head) - sum(tail)
    assert mid > 0 and mid % tf == 0
    return head + [tf] * (mid // tf) + tail


def make_plan_tile(c, tf, ramp=(1250, 1250, 2500)):
    """Plan for tile-major layout: every entry must stay inside one tf-wide
    DRAM block, so the ramp subdivides the first and last blocks."""
    nt = c // tf
    head = list(ramp)
    tail = list(ramp)[::-1]
    assert sum(head) == tf and sum(tail) == tf
    return head + [tf] * (nt - 2) + tail


def build_program(
    rb=RB,
    c=C,
    tf=TF,
    bufs_io=6,
    bufs_res=4,
    store_engine="scalar",
    t_engine="scalar",
    ramp="1250,1250,2500,5000",
    alternate=0,
    layout=LAYOUT,
    group=1,
):
    """Build the single-core Bass/Tile program (shared by all 8 cores)."""
    alu = mybir.AluOpType
    if isinstance(ramp, str):
        ramp = tuple(int(v) for v in ramp.split(",")) if ramp else ()

    nc = bacc.Bacc("TRN2", target_bir_lowering=False, debug=False)
    if layout == "tile":
        # tile-major DRAM: each tf-wide tile is one contiguous [rb, tf]
        # block (partition stride = tf), so every DMA is a fully
        # sequential 1.28MB HBM access
        nt = c // tf
        assert nt * tf == c
        if ramp and sum(ramp) != tf:
            ramp = (tf // 4, tf // 4, tf // 2)
        plan = make_plan_tile(c, tf, ramp) if ramp else [tf] * nt
        x3 = nc.dram_tensor("x", [nt, rb, tf], BF, kind="ExternalInput").ap()
        y3 = nc.dram_tensor("y", [nt, rb, tf], BF, kind="ExternalOutput").ap()
        slices = []
        col = 0
        for w in plan:
            blk, off = col // tf, col % tf
            assert off + w <= tf
            slices.append((x3[blk][:, off : off + w], y3[blk][:, off : off + w], w))
            col += w
    else:
        plan = make_plan(c, tf, ramp) if ramp else [tf] * (c // tf)
        x = nc.dram_tensor("x", [rb, c], BF, kind="ExternalInput").ap()
        y = nc.dram_tensor("y", [rb, c], BF, kind="ExternalOutput").ap()
        slices = []
        col = 0
        for w in plan:
            slices.append((x[:, col : col + w], y[:, col : col + w], w))
            col += w
    assert sum(plan) == c
    t_in = nc.dram_tensor("t", [rb, 1], F32, kind="ExternalInput").ap()
    tv = nc.dram_tensor("tv", [rb, 1], F32, kind="ExternalOutput").ap()

    t_eng = getattr(nc, t_engine)
    store_eng = getattr(nc, store_engine)

    def margin_chain(tc, sp):
        # ---- per-row target margin: f32 in, f32 out ----
        t = sp.tile([rb, 1], F32)
        t_eng.dma_start(t[:], t_in[:])
        t2 = sp.tile([rb, 1], F32)
        nc.vector.tensor_tensor(out=t2[:], in0=t[:], in1=t[:], op=alu.mult)
        om = sp.tile([rb, 1], F32)
        nc.vector.tensor_scalar(
            out=om[:], in0=t2[:], scalar1=-1.0, scalar2=1.0, op0=alu.mult, op1=alu.add
        )
        st = sp.tile([rb, 1], F32)
        nc.scalar.activation(
            out=st[:], in_=om[:], func=mybir.ActivationFunctionType.Sqrt
        )
        # cos branch: S * (t*cos(m) - sin_theta*sin(m))
        a = sp.tile([rb, 1], F32)
        nc.vector.tensor_scalar(
            out=a[:], in0=t[:], scalar1=COS_M * S, scalar2=None, op0=alu.mult
        )
        bb = sp.tile([rb, 1], F32)
        nc.vector.tensor_scalar(
            out=bb[:], in0=st[:], scalar1=SIN_M * S, scalar2=None, op0=alu.mult
        )
        cosm = sp.tile([rb, 1], F32)
        nc.vector.tensor_tensor(out=cosm[:], in0=a[:], in1=bb[:], op=alu.subtract)
        # alt branch: S * (t - sin(pi-m)*m)
        alt = sp.tile([rb, 1], F32)
        nc.vector.tensor_scalar(
            out=alt[:], in0=t[:], scalar1=SINMM, scalar2=S, op0=alu.subtract, op1=alu.mult
        )
        pred = sp.tile([rb, 1], F32)
        nc.vector.tensor_scalar(
            out=pred[:], in0=t[:], scalar1=THETA, scalar2=None, op0=alu.is_gt
        )
        # final = alt + pred * (cosm - alt)
        d = sp.tile([rb, 1], F32)
        nc.vector.tensor_tensor(out=d[:], in0=cosm[:], in1=alt[:], op=alu.subtract)
        pd = sp.tile([rb, 1], F32)
        nc.vector.tensor_tensor(out=pd[:], in0=pred[:], in1=d[:], op=alu.mult)
        final = sp.tile([rb, 1], F32)
        nc.vector.tensor_tensor(out=final[:], in0=alt[:], in1=pd[:], op=alu.add)
        t_eng.dma_start(tv[:], final[:])

    with tile.TileContext(nc) as tc:
        with (
            tc.tile_pool(name="io", bufs=bufs_io) as io_pool,
            tc.tile_pool(name="res", bufs=bufs_res) as res_pool,
            tc.tile_pool(name="small", bufs=1) as sp,
        ):
            # group consecutive full-width slices: one wide load feeds
            # `group` compute/store sub-tiles (row layout only)
            groups = []
            i = 0
            while i < len(slices):
                if (
                    group > 1
                    and layout == "row"
                    and slices[i][2] == tf
                    and i + group <= len(slices)
                    and all(slices[i + k][2] == tf for k in range(group))
                ):
                    groups.append(slices[i : i + group])
                    i += group
                else:
                    groups.append(slices[i : i + 1])
                    i += 1

            # ---- main elementwise pass: out = (x <= T_DEV) ? S*x : 0 ----
            gcol = 0
            for j, grp in enumerate(groups):
                gw = sum(g[2] for g in grp)
                if alternate:
                    # each HWDGE ring carries a load+store mix so store
                    # packets interleave with load packets on every ring
                    load_eng = nc.sync if j % 2 == 0 else store_eng
                    st_eng = store_eng if j % 2 == 0 else nc.sync
                else:
                    load_eng = nc.sync
                    st_eng = store_eng
                xin = io_pool.tile([rb, gw], BF, tag="t")
                if len(grp) == 1:
                    load_eng.dma_start(xin[:], grp[0][0])
                else:
                    load_eng.dma_start(xin[:], x[:, gcol : gcol + gw])
                off = 0
                for xs, ys, w in grp:
                    m = res_pool.tile([rb, w], BF, tag="t")
                    nc.vector.tensor_scalar(
                        out=m[:], in0=xin[:, off : off + w], scalar1=float(T_DEV),
                        scalar2=S, op0=alu.is_le, op1=alu.mult,
                    )
                    nc.vector.tensor_tensor(
                        out=m[:], in0=xin[:, off : off + w], in1=m[:], op=alu.mult
                    )
                    st_eng.dma_start(ys, m[:])
                    off += w
                gcol += gw
                if j == 0:
                    # traced after tile 0 so the ACT Sqrt table load and the
                    # [rb,1] DVE chain overlap the streaming pipeline instead
                    # of delaying the first tile load
                    margin_chain(tc, sp)

    nc.compile()
    return nc


_cached = {}


def _get_program():
    if "nc" not in _cached:
        _cached["nc"] = build_program()
    return _cached["nc"]


def encode_bf16(logits):
    """bf16-quantize the full logits, preserving the f32 (x > 0.3) predicate
    against the device's (x <= T_DEV) compare."""
    xb = logits.astype(BF16)
    xf = xb.astype(np.float32)
    # keep-side violations: x <= 0.3 in f32 but quantized above T_DEV
    viol = (logits <= THRESH) & (xf > T_DEV)
    if viol.any():
        xb[viol] = BF16(T_DEV)
    # dirty-side violations cannot occur (x > 0.3 always rounds to >= 0.30078125)
    return xb


def make_in_maps(logits, labels, layout=LAYOUT, tf=TF):
    logits = np.asarray(logits, dtype=np.float32)
    labels_i = np.asarray(labels).astype(np.int64)
    assert logits.shape == (B, C), logits.shape

    xb = encode_bf16(logits)
    tg = logits[np.arange(B), labels_i].astype(np.float32)

    in_maps = []
    for i in range(N_CORES):
        sl = slice(i * RB, (i + 1) * RB)
        xs = xb[sl]
        if layout == "tile":
            xs = xs.reshape(RB, C // tf, tf).transpose(1, 0, 2)
        in_maps.append(
            {
                "x": np.ascontiguousarray(xs),
                "t": np.ascontiguousarray(tg[sl]).reshape(RB, 1),
            }
        )
    return in_maps


def gather_out(res, labels, layout=LAYOUT, tf=TF):
    labels_i = np.asarray(labels).astype(np.int64)
    ys = []
    for i in range(N_CORES):
        yi = np.asarray(res.results[i]["y"])
        if layout == "tile":
            yi = yi.transpose(1, 0, 2).reshape(RB, C)
        ys.append(yi)
    out = np.concatenate(ys, axis=0).astype(np.float32)
    tv = np.concatenate(
        [np.asarray(res.results[i]["tv"]).reshape(RB) for i in range(N_CORES)]
    ).astype(np.float32)
    out[np.arange(B), labels_i] = tv
    return out


def kernel(logits, labels):
    nc = _get_program()
    in_maps = make_in_maps(logits, labels)
    res = run_bass_kernel_spmd(nc, in_maps, core_ids=list(range(N_CORES)))
    return gather_out(res, labels)


# revision 23
# speedup vs baseline: 1.0075x; 1.0075x over previous
"""CombinedMarginLoss (ArcFace m1=1, m2=0.5, m3=0 + interclass filtering) on 8 trn2 cores.

Sharding: batch dim B=1024 split into 8 slabs of 128 rows (one per core), so
every row's target entry is local to the core that owns the row.

The kernel is HBM-bandwidth bound (pure streaming elementwise over
[1024, 100000] f32), so device I/O uses bf16 to halve the traffic:

- Input encode (host, part of sharding): x_bf16 = round(x). The reference
  predicate (x > 0.3 in f32) can flip under bf16 rounding for x in
  (0.29980, 0.3], so those elements are nudged down to T_DEV = 0.298828125
  (the largest bf16 <= 0.3, exactly representable in both bf16 and f32).
  The device compares against T_DEV, which then reproduces the f32
  predicate exactly; the nudge keeps |xb - x| <= 2 ulp (~0.4% rel).
- The ArcFace margin needs the f32 target logit (sqrt(1-t^2) cancels
  catastrophically near t=1 in bf16), so the host passes the 128 gathered
  target values per core as a tiny f32 side input; the device computes the
  margin chain in f32 and returns it as a small f32 output that the host
  scatters into the final f32 result during unshard.

Per-core program (SPMD, same BIR on all 8 cores):
  - elementwise over [128, 100000] bf16: out = (x <= T_DEV) ? 64*x : 0
    (tensor_scalar mask runs in 4x DVE mode, tensor_tensor mult in 2x)
  - margin chain on [128, 1] f32 from the target-value input, stored to a
    [128, 1] f32 output.
"""

import math

import numpy as np
import ml_dtypes

import concourse.bacc as bacc
import concourse.mybir as mybir
import concourse.tile as tile
from concourse.bass_utils import run_bass_kernel_spmd

B, C = 1024, 100000
N_CORES = 8
RB = B // N_CORES  # 128 rows per core == SBUF partition count

S = 64.0
M2 = 0.5
COS_M = math.cos(M2)
SIN_M = math.sin(M2)
THETA = math.cos(math.pi - M2)
SINMM = math.sin(math.pi - M2) * M2

THRESH = np.float32(0.3)  # the reference's f32 predicate constant
BF16 = ml_dtypes.bfloat16
T_DEV = np.float32(0.298828125)  # largest bf16 <= 0.3; bf16- and f32-exact

TF = 5000  # free-dim tile width (10KB/partition per bf16 tile)
LAYOUT = "row"  # "row" or "tile" (tile-major contiguous DRAM blocks)

F32 = mybir.dt.float32
BF = mybir.dt.bfloat16


def make_plan(c, tf, ramp=(1250, 1250, 2500, 5000)):
    """Tile widths: geometric ramp at both ends so the pipeline fills fast
    (small first load -> compute starts early) and drains fast (small last
    store), full-width tiles in the middle."""
    head = list(ramp)
    tail = list(ramp)[::-1]
    mid = c - sum(head) - sum(tail)
    assert mid > 0 and mid % tf == 0
    return head + [tf] * (mid // tf) + tail


def make_plan_tile(c, tf, ramp=(1250, 1250, 2500)):
    """Plan for tile-major layout: every entry must stay inside one tf-wide
    DRAM block, so the ramp subdivides the first and last blocks."""
    nt = c // tf
    head = list(ramp)
    tail = list(ramp)[::-1]
    assert sum(head) == tf and sum(tail) == tf
    return head + [tf] * (nt - 2) + tail


def build_program(
    rb=RB,
    c=C,
    tf=TF,
    bufs_io=6,
    bufs_res=4,
    store_engine="scalar",
    t_engine="scalar",
    ramp="1250,1250,2500,5000",
    alternate=0,
    layout=LAYOUT,
    group=1,
    fill2=1,
    drain2=0,
):
    """Build the single-core Bass/Tile program (shared by all 8 cores)."""
    alu = mybir.AluOpType
    if isinstance(ramp, str):
        ramp = tuple(int(v) for v in ramp.split(",")) if ramp else ()

    nc = bacc.Bacc("TRN2", target_bir_lowering=False, debug=False)
    if layout == "tile":
        # tile-major DRAM: each tf-wide tile is one contiguous [rb, tf]
        # block (partition stride = tf), so every DMA is a fully
        # sequential 1.28MB HBM access
        nt = c // tf
        assert nt * tf == c
        if ramp and sum(ramp) != tf:
            ramp = (tf // 4, tf // 4, tf // 2)
        plan = make_plan_tile(c, tf, ramp) if ramp else [tf] * nt
        x3 = nc.dram_tensor("x", [nt, rb, tf], BF, kind="ExternalInput").ap()
        y3 = nc.dram_tensor("y", [nt, rb, tf], BF, kind="ExternalOutput").ap()
        slices = []
        col = 0
        for w in plan:
            blk, off = col // tf, col % tf
            assert off + w <= tf
            slices.append((x3[blk][:, off : off + w], y3[blk][:, off : off + w], w))
            col += w
    else:
        plan = make_plan(c, tf, ramp) if ramp else [tf] * (c // tf)
        x = nc.dram_tensor("x", [rb, c], BF, kind="ExternalInput").ap()
        y = nc.dram_tensor("y", [rb, c], BF, kind="ExternalOutput").ap()
        slices = []
        col = 0
        for w in plan:
            slices.append((x[:, col : col + w], y[:, col : col + w], w))
            col += w
    assert sum(plan) == c
    t_in = nc.dram_tensor("t", [rb, 1], F32, kind="ExternalInput").ap()
    tv = nc.dram_tensor("tv", [rb, 1], F32, kind="ExternalOutput").ap()

    t_eng = getattr(nc, t_engine)
    store_eng = getattr(nc, store_engine)

    def margin_chain(tc, sp):
        # ---- per-row target margin: f32 in, f32 out ----
        t = sp.tile([rb, 1], F32)
        t_eng.dma_start(t[:], t_in[:])
        t2 = sp.tile([rb, 1], F32)
        nc.vector.tensor_tensor(out=t2[:], in0=t[:], in1=t[:], op=alu.mult)
        om = sp.tile([rb, 1], F32)
        nc.vector.tensor_scalar(
            out=om[:], in0=t2[:], scalar1=-1.0, scalar2=1.0, op0=alu.mult, op1=alu.add
        )
        st = sp.tile([rb, 1], F32)
        nc.scalar.activation(
            out=st[:], in_=om[:], func=mybir.ActivationFunctionType.Sqrt
        )
        # cos branch: S * (t*cos(m) - sin_theta*sin(m))
        a = sp.tile([rb, 1], F32)
        nc.vector.tensor_scalar(
            out=a[:], in0=t[:], scalar1=COS_M * S, scalar2=None, op0=alu.mult
        )
        bb = sp.tile([rb, 1], F32)
        nc.vector.tensor_scalar(
            out=bb[:], in0=st[:], scalar1=SIN_M * S, scalar2=None, op0=alu.mult
        )
        cosm = sp.tile([rb, 1], F32)
        nc.vector.tensor_tensor(out=cosm[:], in0=a[:], in1=bb[:], op=alu.subtract)
        # alt branch: S * (t - sin(pi-m)*m)
        alt = sp.tile([rb, 1], F32)
        nc.vector.tensor_scalar(
            out=alt[:], in0=t[:], scalar1=SINMM, scalar2=S, op0=alu.subtract, op1=alu.mult
        )
        pred = sp.tile([rb, 1], F32)
        nc.vector.tensor_scalar(
            out=pred[:], in0=t[:], scalar1=THETA, scalar2=None, op0=alu.is_gt
        )
        # final = alt + pred * (cosm - alt)
        d = sp.tile([rb, 1], F32)
        nc.vector.tensor_tensor(out=d[:], in0=cosm[:], in1=alt[:], op=alu.subtract)
        pd = sp.tile([rb, 1], F32)
        nc.vector.tensor_tensor(out=pd[:], in0=pred[:], in1=d[:], op=alu.mult)
        final = sp.tile([rb, 1], F32)
        nc.vector.tensor_tensor(out=final[:], in0=alt[:], in1=pd[:], op=alu.add)
        t_eng.dma_start(tv[:], final[:])

    with tile.TileContext(nc) as tc:
        with (
            tc.tile_pool(name="io", bufs=bufs_io) as io_pool,
            tc.tile_pool(name="res", bufs=bufs_res) as res_pool,
            tc.tile_pool(name="small", bufs=1) as sp,
        ):
            # group consecutive full-width slices: one wide load feeds
            # `group` compute/store sub-tiles (row layout only)
            groups = []
            i = 0
            while i < len(slices):
                if (
                    group > 1
                    and layout == "row"
                    and slices[i][2] == tf
                    and i + group <= len(slices)
                    and all(slices[i + k][2] == tf for k in range(group))
                ):
                    groups.append(slices[i : i + group])
                    i += group
                else:
                    groups.append(slices[i : i + 1])
                    i += 1

            # ---- main elementwise pass: out = (x <= T_DEV) ? S*x : 0 ----
            gcol = 0
            for j, grp in enumerate(groups):
                gw = sum(g[2] for g in grp)
                if alternate:
                    # each HWDGE ring carries a load+store mix so store
                    # packets interleave with load packets on every ring
                    load_eng = nc.sync if j % 2 == 0 else store_eng
                    st_eng = store_eng if j % 2 == 0 else nc.sync
                else:
                    # fill2: second ramp load rides the store ring so the
                    # first two loads land in parallel during pipeline fill;
                    # drain2: the last two stores ride the (by then idle)
                    # load ring so the tail drains on both rings
                    load_eng = store_eng if (fill2 and j == 1) else nc.sync
                    st_eng = (
                        nc.sync
                        if (drain2 and j >= len(groups) - drain2)
                        else store_eng
                    )
                xin = io_pool.tile([rb, gw], BF, tag="t")
                if len(grp) == 1:
                    load_eng.dma_start(xin[:], grp[0][0])
                else:
                    load_eng.dma_start(xin[:], x[:, gcol : gcol + gw])
                off = 0
                for xs, ys, w in grp:
                    m = res_pool.tile([rb, w], BF, tag="t")
                    nc.vector.tensor_scalar(
                        out=m[:], in0=xin[:, off : off + w], scalar1=float(T_DEV),
                        scalar2=S, op0=alu.is_le, op1=alu.mult,
                    )
                    nc.vector.tensor_tensor(
                        out=m[:], in0=xin[:, off : off + w], in1=m[:], op=alu.mult
                    )
                    st_eng.dma_start(ys, m[:])
                    off += w
                gcol += gw
                if j == 0:
                    # traced after tile 0 so the ACT Sqrt table load and the
                    # [rb,1] DVE chain overlap the streaming pipeline instead
                    # of delaying the first tile load
                    margin_chain(tc, sp)

    nc.compile()
    return nc


_cached = {}


def _get_program():
    if "nc" not in _cached:
        _cached["nc"] = build_program()
    return _cached["nc"]


def encode_bf16(logits):
    """bf16-quantize the full logits, preserving the f32 (x > 0.3) predicate
    against the device's (x <= T_DEV) compare."""
    xb = logits.astype(BF16)
    xf = xb.astype(np.float32)
    # keep-side violations: x <= 0.3 in f32 but quantized above T_DEV
    viol = (logits <= THRESH) & (xf > T_DEV)
    if viol.any():
        xb[viol] = BF16(T_DEV)
    # dirty-side violations cannot occur (x > 0.3 always rounds to >= 0.30078125)
    return xb


def make_in_maps(logits, labels, layout=LAYOUT, tf=TF):
    logits = np.asarray(logits, dtype=np.float32)
    labels_i = np.asarray(labels).astype(np.int64)
    assert logits.shape == (B, C), logits.shape

    xb = encode_bf16(logits)
    tg = logits[np.arange(B), labels_i].astype(np.float32)

    in_maps = []
    for i in range(N_CORES):
        sl = slice(i * RB, (i + 1) * RB)
        xs = xb[sl]
        if layout == "tile":
            xs = xs.reshape(RB, C // tf, tf).transpose(1, 0, 2)
        in_maps.append(
            {
                "x": np.ascontiguousarray(xs),
                "t": np.ascontiguousarray(tg[sl]).reshape(RB, 1),
            }
        )
    return in_maps


def gather_out(res, labels, layout=LAYOUT, tf=TF):
    labels_i = np.asarray(labels).astype(np.int64)
    ys = []
    for i in range(N_CORES):
        yi = np.asarray(res.results[i]["y"])
        if layout == "tile":
            yi = yi.transpose(1, 0, 2).reshape(RB, C)
        ys.append(yi)
    out = np.concatenate(ys, axis=0).astype(np.float32)
    tv = np.concatenate(
        [np.asarray(res.results[i]["tv"]).reshape(RB) for i in range(N_CORES)]
    ).astype(np.float32)
    out[np.arange(B), labels_i] = tv
    return out


def kernel(logits, labels):
    nc = _get_program()
    in_maps = make_in_maps(logits, labels)
    res = run_bass_kernel_spmd(nc, in_maps, core_ids=list(range(N_CORES)))
    return gather_out(res, labels)
